# revision 13
# baseline (speedup 1.0000x reference)
"""MixedScore MultiHeadAttention Trainium2 kernel (8 NeuronCores).

score = ((q + z1) . (k + z2)) / sqrt(d), z1/z2 = per-(n,m) projections of z.
Fused per (b, n): project z[b,n] with W1/W2, add q/k, multiply, segment-sum
over d, softmax over m, weighted sum with v.  z is read exactly once.

Engine split per query row n (the balance that matters):
  PE    : kt preload into psum (I-matmul) + two projections + selector
  Scalar: a_sb = a_ps + q  (psum read, bias fused)
  Vector: p = b_ps * a_sb  (single product)
For 1-in-4 rows the kt preload moves to Vector (b_sb = b_ps + kt) to
rebalance PE vs Vector.  GpSimd does no elementwise work (it is ~2x
slower than DVE per op and was the old bottleneck).

Sharding: 8 cores = (b in {0,1}) x (four 128-row n-chunks).  No collectives.
"""

import sys

sys.path.insert(0, "/opt/trn_rl_repo")

import numpy as np

import concourse.bacc as bacc
import concourse.bass as bass
import concourse.tile as tile
from concourse import mybir
from concourse.bass_utils import run_bass_kernel_spmd

B, H, N, D, E = 2, 8, 512, 16, 128
HD = H * D  # 128
NCHUNK = N // 4  # 128 query rows per core
NB = 16  # query rows per inner batch (16*8 heads = 128 psum rows)
F32 = mybir.dt.float32
F16 = mybir.dt.float16

# fraction of rows whose kt-add rides the PE (rest go to Vector)
VADD_EVERY = 2  # n % VADD_EVERY == VADD_EVERY-1 -> Vector does the k-add
SEL_LAG = 2  # selector matmul lags the projections by this many rows
EXP_BIAS = -6.0  # keeps exp() in fp16 range; softmax shift-invariant

_CACHE = {}


def _build_program(nchunk=NCHUNK):
    nc = bacc.Bacc("TRN2", target_bir_lowering=False, debug=False,
                   enable_asserts=True, num_devices=8)

    zt_d = nc.dram_tensor("zt_s", [nchunk, E, N], F16, kind="ExternalInput")
    qt_d = nc.dram_tensor("qt_s", [HD, nchunk], F32, kind="ExternalInput")
    kt_d = nc.dram_tensor("kt_s", [HD, N], F16, kind="ExternalInput")
    vf_d = nc.dram_tensor("vf_s", [N, HD], F16, kind="ExternalInput")
    w1_d = nc.dram_tensor("w1", [E, HD], F16, kind="ExternalInput")
    w2_d = nc.dram_tensor("w2", [E, HD], F16, kind="ExternalInput")
    ssel_d = nc.dram_tensor("ssel", [HD, NB, 128], F16, kind="ExternalInput")
    ident_d = nc.dram_tensor("ident", [128, 128], F16, kind="ExternalInput")
    identr_d = nc.dram_tensor("identr", [D, D], F16, kind="ExternalInput")
    out_d = nc.dram_tensor("out_s", [nchunk, HD], F32, kind="ExternalOutput")

    nbatch = nchunk // NB
    ntot = nchunk  # flat row count

    with tile.TileContext(nc) as tc:
        from contextlib import ExitStack
        loop_ps = ExitStack()
        with (
            tc.tile_pool(name="consts", bufs=1) as consts,
            tc.tile_pool(name="zt", bufs=4) as ztp,
            tc.tile_pool(name="ab", bufs=4) as abp,
            tc.tile_pool(name="bb", bufs=3) as bbp,
            tc.tile_pool(name="pp", bufs=6) as ppool,
            tc.tile_pool(name="ep", bufs=4) as epool,
        ):
            psA = loop_ps.enter_context(tc.tile_pool(name="psA", bufs=2, space="PSUM"))
            psB = loop_ps.enter_context(tc.tile_pool(name="psB", bufs=3, space="PSUM"))
            psS = loop_ps.enter_context(tc.tile_pool(name="psS", bufs=2, space="PSUM"))
            psO = loop_ps.enter_context(tc.tile_pool(name="psO", bufs=1, space="PSUM"))
            # consts needed by the first rows go first on the queue
            w1_sb = consts.tile([E, HD], F16, tag="w1")
            nc.gpsimd.dma_start(out=w1_sb[:], in_=w1_d.ap())
            w2_sb = consts.tile([E, HD], F16, tag="w2")
            nc.gpsimd.dma_start(out=w2_sb[:], in_=w2_d.ap())
            ident_sb = consts.tile([128, 128], F16, tag="ident")
            nc.gpsimd.dma_start(out=ident_sb[:], in_=ident_d.ap())
            kt_sb = consts.tile([HD, N], F16, tag="kt")
            nc.gpsimd.dma_start(out=kt_sb[:], in_=kt_d.ap())
            qt_sb = consts.tile([HD, nchunk], F32, tag="qt")
            nc.gpsimd.dma_start(out=qt_sb[:], in_=qt_d.ap())
            ssel_sb = consts.tile([HD, NB, 128], F16, tag="ssel")
            nc.gpsimd.dma_start(out=ssel_sb[:], in_=ssel_d.ap())
            identr_sb = consts.tile([D, D], F16, tag="identr")
            nc.gpsimd.dma_start(out=identr_sb[:], in_=identr_d.ap())
            # v in m-partition layout: [m-in-tile, mtile, hd]
            vf_sb = consts.tile([128, 4, HD], F16, tag="vf")
            nc.gpsimd.dma_start(
                out=vf_sb[:], in_=vf_d.ap().rearrange("(t p) c -> p t c", p=128)
            )

            et_all = consts.tile([128, 4, nbatch, NB, H], F16, tag="et_all")
            ebias_sb = consts.tile([128, 1], F32, tag="ebias")
            nc.gpsimd.memset(ebias_sb[:], EXP_BIAS)

            # ---- flat pipelined main loop over all query rows ----
            zts = {}  # (j, quad) -> tile
            score_tiles = {}
            p_tiles = {}

            def fetch_batch(j):
                if j >= nbatch:
                    return
                zsl = zt_d[j * NB:(j + 1) * NB].rearrange("n e m -> e n m")
                for q4 in range(4):
                    zq = ztp.tile([E, 4, N], F16, tag=f"zt{q4}",
                                  name=f"zt{j}_{q4}")
                    nc.sync.dma_start(
                        out=zq[:], in_=zsl[:, q4 * 4:(q4 + 1) * 4, :]
                    )
                    zts[(j, q4)] = zq

            fetch_batch(0)
            fetch_batch(1)

            def emit_row(g):
                """projections + elementwise for flat row g"""
                j, n = g // NB, g % NB
                if n == 0 and j + 2 <= nbatch - 1 + 2:
                    fetch_batch(j + 2)
                if n == 0:
                    score_tiles[j] = psS.tile([128, N], F32, tag="score",
                                              name=f"score{j}")
                zcol = zts[(j, n // 4)][:, n % 4, :]  # [e=128, m=512]
                b_ps = psB.tile([HD, N], F32, tag="b")
                vadd = (n % VADD_EVERY == VADD_EVERY - 1)
                if vadd:
                    nc.tensor.matmul(b_ps[:], w2_sb[:], zcol)
                else:
                    # kt preload: b_ps = I^T @ kt = kt, then += W2 @ z
                    nc.tensor.matmul(b_ps[:], ident_sb[:], kt_sb[:],
                                     start=True, stop=False)
                    nc.tensor.matmul(b_ps[:], w2_sb[:], zcol,
                                     start=False, stop=True)
                a_ps = psA.tile([HD, N], F32, tag="a")
                nc.tensor.matmul(a_ps[:], w1_sb[:], zcol)

                qcol = qt_sb[:, g:g + 1]
                a_sb = abp.tile([HD, N], F16, tag="a_sb")
                nc.scalar.add(a_sb[:], a_ps[:], qcol)  # S: z1 + q -> fp16

                p_sb = ppool.tile([HD, N], F16, tag="p")
                if vadd:
                    b_sb = bbp.tile([HD, N], F16, tag="b_sb")
                    nc.vector.tensor_add(b_sb[:], b_ps[:], kt_sb[:])
                    nc.vector.tensor_mul(p_sb[:], a_sb[:], b_sb[:])
                else:
                    nc.vector.tensor_mul(p_sb[:], b_ps[:], a_sb[:])
                p_tiles[g] = p_sb

            def emit_sel(g):
                j, n = g // NB, g % NB
                nc.tensor.matmul(
                    score_tiles[j][:], ssel_sb[:, n, :], p_tiles.pop(g)[:],
                    start=(n == 0), stop=(n == NB - 1),
                )

            def emit_softmax(j):
                score_ps = score_tiles.pop(j)
                e_sb = epool.tile([128, N], F16, tag="e")
                rowsum = epool.tile([128, 1], F32, tag="rowsum")
                nc.scalar.activation(
                    e_sb[:], score_ps[:], func=mybir.ActivationFunctionType.Exp,
                    bias=ebias_sb[:], accum_out=rowsum[:],
                )
                rinv = epool.tile([128, 1], F32, tag="rinv")
                nc.vector.reciprocal(rinv[:], rowsum[:])
                en_sb = epool.tile([128, N], F16, tag="en")
                nc.scalar.mul(en_sb[:], e_sb[:], mul=rinv[:])
                # transpose each 128-col m-block via the DMA XBAR (no PE, no
                # psum bank, no copy): et_all[:, t, j, :, :] = en[:, t-block]^T
                for t in range(4):
                    nc.sync.dma_start_transpose(
                        out=et_all[:, t, j, :, :].rearrange("p a b -> p (a b)"),
                        in_=en_sb[:, t * 128:(t + 1) * 128],
                    )

            # out accumulator: o_all[d, h, (j n)] = sum_m v[m, hd] w[(j n), h, m]
            # written per-batch as its et tiles land (16-col accumulation
            # groups); one half-width psum tile per 4 batches, copied to
            # SBUF mid-loop so only batch 7's slice remains for the tail.
            half = nbatch // 2
            oall_sb = epool.tile([D, H, nchunk], F16, tag="oall_sb")
            o_half = {}

            def emit_out(j):
                jh, jr = j // half, j % half
                if jr == 0:
                    o_half[jh] = psO.tile([D, H, half * NB], F32, tag="o",
                                          name=f"o_{jh}")
                cols = slice(jr * NB, (jr + 1) * NB)
                for h in range(H):
                    for mt in range(4):
                        nc.tensor.matmul(
                            o_half[jh][:, h, cols],
                            vf_sb[:, mt, h * D:(h + 1) * D],
                            et_all[:, mt, j, :, h],
                            start=(mt == 0), stop=(mt == 3),
                        )
                if jr == half - 1:
                    nc.vector.tensor_copy(
                        out=oall_sb[:, :, jh * half * NB:(jh + 1) * half * NB],
                        in_=o_half.pop(jh)[:],
                    )

            OUT_DEFER = 6  # rows between a batch's et DMAs and its out matmuls
            for g in range(ntot + SEL_LAG + OUT_DEFER + 1):
                if g < ntot:
                    emit_row(g)
                if 0 <= g - SEL_LAG < ntot:
                    emit_sel(g - SEL_LAG)
                    jd, nd = (g - SEL_LAG) // NB, (g - SEL_LAG) % NB
                    if nd == NB - 1:
                        emit_softmax(jd)
                go = g - SEL_LAG - OUT_DEFER
                if go >= 0 and go % NB == NB - 1:
                    emit_out(go // NB)

            loop_ps.close()
            _psF_cm = tc.tile_pool(name="psF", bufs=1, space="PSUM")
            psF = _psF_cm.__enter__()
            # transpose [d, n-chunk] blocks back to [n, h*16+d]
            fin_ps = psF.tile([nchunk, HD], F16, tag="fin", name="fin_ps")
            for h in range(H):
                nc.tensor.transpose(
                    fin_ps[:, h * D:(h + 1) * D],
                    oall_sb[:, h, :],
                    identr_sb[:],
                )
            fin_sb = epool.tile([nchunk, HD], F32, tag="fin_sb")
            nc.vector.tensor_copy(out=fin_sb[:], in_=fin_ps[:])
            nc.sync.dma_start(out=out_d[:], in_=fin_sb[:])
            _psF_cm.__exit__(None, None, None)

    nc.compile()
    return nc


def _get_program(nchunk=NCHUNK):
    key = nchunk
    if key not in _CACHE:
        _CACHE[key] = _build_program(nchunk)
    return _CACHE[key]


def _prep_shards(q, k, v, z, Wz1, Wz2):
    q = np.asarray(q, np.float32)
    k = np.asarray(k, np.float32)
    v = np.asarray(v, np.float32)
    z = np.asarray(z, np.float32)
    # [B, N, E, M]: per (b,n) a [e, m] block, contiguous
    zt = np.ascontiguousarray(z.transpose(0, 1, 3, 2)).astype(np.float16)
    qt = np.ascontiguousarray(q.transpose(0, 1, 3, 2).reshape(B, HD, N))
    kt = np.ascontiguousarray(k.transpose(0, 1, 3, 2).reshape(B, HD, N))
    vf = np.ascontiguousarray(v.transpose(0, 2, 1, 3).reshape(B, N, HD)).astype(np.float16)
    ssel = np.zeros((HD, NB, 128), np.float16)
    for n in range(NB):
        for h in range(H):
            ssel[h * D:(h + 1) * D, n, n * H + h] = 1.0 / np.sqrt(D)
    ident = np.eye(128, dtype=np.float16)
    identr = np.eye(D, dtype=np.float16)
    w1 = np.ascontiguousarray(np.asarray(Wz1, np.float16))
    w2 = np.ascontiguousarray(np.asarray(Wz2, np.float16))

    in_maps = []
    for c in range(8):
        b, nt = c // 4, c % 4
        n0 = nt * NCHUNK
        in_maps.append({
            "zt_s": np.ascontiguousarray(zt[b, n0:n0 + NCHUNK]),
            "qt_s": np.ascontiguousarray(qt[b, :, n0:n0 + NCHUNK]),
            "kt_s": kt[b].astype(np.float16),
            "vf_s": vf[b],
            "w1": w1,
            "w2": w2,
            "ssel": ssel,
            "ident": ident,
            "identr": identr,
        })
    return in_maps


def _run(inputs, trace=False, trace_kwargs=None):
    nc = _get_program()
    in_maps = _prep_shards(inputs["q"], inputs["k"], inputs["v"],
                           inputs["z"], inputs["Wz1"], inputs["Wz2"])
    res = run_bass_kernel_spmd(
        nc, in_maps, core_ids=list(range(8)), trace=trace,
        **(trace_kwargs or {}),
    )
    out = np.empty((B, N, HD), np.float32)
    for c in range(8):
        b, nt = c // 4, c % 4
        out[b, nt * NCHUNK:(nt + 1) * NCHUNK, :] = res.results[c]["out_s"]
    return out, res


def kernel(**inputs):
    out, _ = _run(inputs, trace=False)
    return out
